# revision 7
# baseline (speedup 1.0000x reference)
"""Trainium2 Bass kernel for nn_Block_with_lora (dense transformer block).

Sharding: 8 cores = 4 batches x 2 token-parity shards (stride-2 over T).
Each core computes its 512 query tokens end-to-end (no collectives);
K/V projections over all 1024 tokens are computed per-core (uniform SPMD
program; all batch/parity dependence lives in the per-core input data).

Layout: all activations transposed [C, T] (host transposes I/O), so every
projection is a natural PE matmul. Attention uses S^T = K^T.T @ Q^T tiles
[tk, tq]; softmax denominator rides the AV matmul as an extra ones-column
of V; masking = additive diagonal band (DVE) + rectangle memsets (GPSIMD).
"""

import sys

sys.path.insert(0, "/opt/trn_rl_repo")

import numpy as np
import ml_dtypes
from contextlib import ExitStack

BF = ml_dtypes.bfloat16

C = 1024
H = 16
DH = 64
R = 16
SCALE = 1.0 / R
T = 1024
TQ = 512
NT = 8  # C / 128
EPS = 1e-5
NCORES = 8

_PROG = None


def _build_program():
    import concourse.bass as bass
    import concourse.tile as tile
    from concourse import mybir, bacc

    f32 = mybir.dt.float32
    bf16 = mybir.dt.bfloat16
    AF = mybir.ActivationFunctionType
    AL = mybir.AluOpType

    nc = bacc.Bacc("TRN2", target_bir_lowering=False, debug=False)

    def din(name, shape, dt=f32):
        return nc.dram_tensor(name, shape, dt, kind="ExternalInput").ap()

    xT_d = din("xT", [C, T])
    xqT_d = din("xqT", [C, TQ])
    fT_d = din("fT", [C, T])
    band_d = din("band", [128, 64])

    w_d = {}
    for n in ["wq", "wk", "wv", "wsp", "wcq", "wck", "wcv", "wcp"]:
        w_d[n] = din(n, [C, C], bf16)
    w_d["wfc"] = din("wfc", [C, 4 * C], bf16)
    w_d["wpr"] = din("wpr", [4 * C, C], bf16)
    a_d = {n: din(n, [C, R], bf16) for n in ["a_sa", "a_sp", "a_cq", "a_ck", "a_cp"]}
    b_d = {
        n: din(n, [R, C], bf16)
        for n in ["b_saq", "b_sak", "b_sav", "b_sp", "b_cq", "b_ckk", "b_ckv", "b_cp"]
    }
    bias_d = {
        n: din(n, [C], f32)
        for n in ["bq", "bk", "bsp", "bcq", "bck", "bcp", "bpr", "g1", "b1", "g2", "b2"]
    }
    bias_d["bfc"] = din("bfc", [4 * C], f32)
    bvrow_d = din("bv_row", [1, C], bf16)
    bcvrow_d = din("bcv_row", [1, C], bf16)

    outT_d = nc.dram_tensor("outT", [C, TQ], f32, kind="ExternalOutput").ap()

    with tile.TileContext(nc) as tc, ExitStack() as ctx:

        def pool(name, bufs, space=None):
            kw = dict(name=name, bufs=bufs)
            if space:
                kw["space"] = space
            return ctx.enter_context(tc.tile_pool(**kw))

        # SBUF pools (budget ~181KB/partition of 192)
        big32 = pool("big32", 3)        # [128,1024] f32: x/f stream + LN temps
        acts = pool("acts", 8)          # [128,1024] bf16: lnb then fb
        lnsm = pool("lnsm", 8)          # [128,512] bf16: lnown -> ln1b -> ln2
        qpool = pool("qpool", 8)        # [128,512] bf16: qT -> q2T
        kpool = pool("kpool", 8)        # [128,1024] bf16: kT
        k2pool = pool("k2pool", 8)      # [128,1024] bf16: k2T (separate: overlaps attn)
        vpool = pool("vpool", 8)        # [128,1040] bf16: V -> V2
        opool = pool("opool", 8)        # [128,512] bf16: oT -> o2T
        rpool = pool("rpool", 8)        # [128,512] f32: residual (persist)
        mpool = pool("mpool", 32)       # [128,256] bf16: MLP hidden (per t-half)
        wpool = pool("wpool", 8)        # [128,512] bf16: weight chunks
        epool = pool("epool", 2)        # [128,1024] bf16: exp(S)
        sqpool = pool("sqpool", 2)      # squares for LN var
        sbig = pool("sbig", 2)          # [128,1024] f32: LN mean/rstd bcast
        rows = pool("rows", 2)          # [1,1024] f32: LN stat rows
        rrows = pool("rrows", 2)        # [1,512] f32: softmax recip rows
        recb = pool("recb", 2)          # [64,512] f32: recip bcast
        outfp = pool("outfp", 2)        # [128,256] f32: final out staging
        zpool = pool("zpool", 1)        # [16,*] bf16: lora z (1 slot per tag)
        lorab = pool("lorab", 1)        # [16,1024] bf16: lora B rows
        loraa = pool("loraa", 10)       # [128,16] bf16: lora A chunks
        smalls = pool("smalls", 1)      # [128,<=32] bias/g/b columns (per tag)
        onesp = pool("onesp", 1)
        bandp = pool("bandp", 1)
        bvp = pool("bvp", 1)            # [1,1024] bf16 v-bias rows

        # PSUM pools: 4 + 2 + 2 = 8 banks
        ps = pool("ps", 2, space="PSUM")   # [128,1024] f32: S tiles, LN stats, pr acc
        po = pool("po", 2, space="PSUM")   # [65..128,512] f32: attn out acc, pr acc
        pp = pool("pp", 2, space="PSUM")   # [128,512] f32: projections, z

        # ---- constants ----
        ones_c32 = onesp.tile([128, 1], f32, tag="oc32")
        nc.gpsimd.memset(ones_c32[:], 1.0)
        ones_c16 = onesp.tile([128, 1], bf16, tag="oc16")
        nc.gpsimd.memset(ones_c16[:], 1.0)
        ones_r16 = onesp.tile([1, 128], bf16, tag="or16")
        nc.gpsimd.memset(ones_r16[:], 1.0)
        ones_r32 = onesp.tile([1, 128], f32, tag="or32")
        nc.gpsimd.memset(ones_r32[:], 1.0)

        band_t = bandp.tile([128, 64], f32, tag="band")
        nc.sync.dma_start(band_t[:], band_d[:, :])
        eps_t = onesp.tile([1, 1], f32, tag="eps")
        nc.gpsimd.memset(eps_t[:], EPS)

        def load_percol(name, n=NT):
            t = smalls.tile([128, n], f32, tag=name)
            nc.sync.dma_start(t[:], bias_d[name].rearrange("(m p) -> p m", p=128))
            return t

        bias_t = {
            n: load_percol(n)
            for n in ["bq", "bk", "bsp", "bcq", "bcp", "bpr", "g1", "b1", "g2", "b2", "bck"]
        }
        bias_t["bfc"] = load_percol("bfc", 32)
        bv_t = bvp.tile([1, C], bf16, tag="bv")
        nc.sync.dma_start(bv_t[:], bvrow_d[:, :])
        bcv_t = bvp.tile([1, C], bf16, tag="bcv")
        nc.sync.dma_start(bcv_t[:], bcvrow_d[:, :])

        def load_lora_a(name):
            ts = []
            for k in range(NT):
                t = loraa.tile([128, R], bf16, tag="loraa")
                nc.sync.dma_start(t[:], a_d[name][k * 128:(k + 1) * 128, :])
                ts.append(t)
            return ts

        def load_lora_b(name):
            t = lorab.tile([R, C], bf16, tag="lorab")
            nc.sync.dma_start(t[:], b_d[name][:, :])
            return t

        # =============== helpers ===============
        def bcast_row(row, out_sb, Tn):
            # broadcast [1, Tn] f32 row to [128, Tn] SBUF via K=1 PE matmul
            for h in range(Tn // 512):
                sl = slice(h * 512, (h + 1) * 512)
                bp = pp.tile([128, 512], f32, tag="pp")
                nc.tensor.matmul(bp[:], ones_r32[:], row[0:1, sl], start=True, stop=True)
                nc.vector.tensor_copy(out_sb[:, sl], bp[:])

        def ln_stats_and_norm(src_tiles, g_col, b_col, out_tiles):
            """LayerNorm over channel (partition) dim; src 8x[128,512] f32 persistent."""
            mean_ps = ps.tile([1, TQ], f32, tag="ps")
            sq_ps = ps.tile([1, TQ], f32, tag="ps")
            for k in range(NT):
                sq = sqpool.tile([128, TQ], bf16, tag="sqo")
                nc.vector.tensor_mul(sq[:], src_tiles[k][:], src_tiles[k][:])
                nc.tensor.matmul(mean_ps[:], ones_c32[:], src_tiles[k][:],
                                 start=(k == 0), stop=(k == NT - 1))
                nc.tensor.matmul(sq_ps[:], ones_c16[:], sq[:],
                                 start=(k == 0), stop=(k == NT - 1))
            mean_row = rows.tile([1, TQ], f32, tag="rows")
            rstd_row = rows.tile([1, TQ], f32, tag="rows")
            nc.vector.tensor_scalar_mul(mean_row[:], mean_ps[:], 1.0 / C)
            nc.vector.tensor_mul(rstd_row[:], mean_row[:], mean_row[:])
            nc.vector.scalar_tensor_tensor(rstd_row[:], sq_ps[:], 1.0 / C, rstd_row[:],
                                           op0=AL.mult, op1=AL.subtract)
            nc.scalar.activation(rstd_row[:], rstd_row[:], AF.Sqrt, bias=eps_t[:])
            nc.vector.reciprocal(rstd_row[:], rstd_row[:])
            mb = sbig.tile([128, TQ], f32, tag="sbig")
            rb = sbig.tile([128, TQ], f32, tag="sbig")
            bcast_row(mean_row, mb, TQ)
            bcast_row(rstd_row, rb, TQ)
            for k in range(NT):
                t1 = big32.tile([128, TQ], f32, tag="big32")
                nc.vector.tensor_sub(t1[:], src_tiles[k][:], mb[:])
                nc.vector.tensor_mul(t1[:], t1[:], rb[:])
                nc.scalar.activation(out_tiles[k][:], t1[:], AF.Identity,
                                     bias=b_col[:, k:k + 1], scale=g_col[:, k:k + 1])

        def compute_z(a_tiles, rhs_tiles, Tn, tag):
            """z^T = A-proj of activations: [16, Tn] bf16."""
            z_sb = zpool.tile([R, Tn], bf16, tag=tag)
            for h in range(Tn // 512):
                sl = slice(h * 512, (h + 1) * 512)
                zp = pp.tile([R, 512], f32, tag="pp")
                for k in range(NT):
                    nc.tensor.matmul(zp[:], a_tiles[k][:], rhs_tiles[k][:, sl],
                                     start=(k == 0), stop=(k == NT - 1))
                nc.vector.tensor_copy(z_sb[:, sl], zp[:])
            return z_sb

        def projT(wname, rhs_tiles, Tn, z_sb, bname, out_cb):
            """out^T tiles via PE; lora + callback per (M-tile, t-half) psum."""
            b_t = load_lora_b(bname)
            for mh in range(2):  # c_out halves of 512
                wts = []
                for k in range(NT):
                    wt = wpool.tile([128, 512], bf16, tag="wpool")
                    nc.sync.dma_start(wt[:], w_d[wname][k * 128:(k + 1) * 128,
                                                        mh * 512:(mh + 1) * 512])
                    wts.append(wt)
                for ml in range(4):
                    mi = mh * 4 + ml
                    for h in range(Tn // 512):
                        sl = slice(h * 512, (h + 1) * 512)
                        pt = pp.tile([128, 512], f32, tag="pp")
                        for k in range(NT):
                            nc.tensor.matmul(pt[:], wts[k][:, ml * 128:(ml + 1) * 128],
                                             rhs_tiles[k][:, sl], start=(k == 0), stop=False)
                        nc.tensor.matmul(pt[:], b_t[:, mi * 128:(mi + 1) * 128],
                                         z_sb[:, sl], start=False, stop=True)
                        out_cb(mi, pt, h)

        def proj_V(wname, lhs_tiles, z_sb, bv_row_t, bname, v_tiles):
            """V natural [t, d] with activations stationary; +lora +bias(ones-MM)."""
            b_t = load_lora_b(bname)
            for dh in range(2):
                sl = slice(dh * 512, (dh + 1) * 512)
                wts = []
                for k in range(NT):
                    wt = wpool.tile([128, 512], bf16, tag="wpool")
                    nc.sync.dma_start(wt[:], w_d[wname][k * 128:(k + 1) * 128, sl])
                    wts.append(wt)
                for tt in range(NT):
                    pt = pp.tile([128, 512], f32, tag="pp")
                    for k in range(NT):
                        nc.tensor.matmul(pt[:], lhs_tiles[k][:, tt * 128:(tt + 1) * 128],
                                         wts[k][:], start=(k == 0), stop=False)
                    nc.tensor.matmul(pt[:], z_sb[:, tt * 128:(tt + 1) * 128],
                                     b_t[:, sl], start=False, stop=False)
                    nc.tensor.matmul(pt[:], ones_r16[:], bv_row_t[:, sl],
                                     start=False, stop=True)
                    dest = v_tiles[tt][:, dh * 520:(dh + 1) * 520]
                    dest = dest.rearrange("p (h d) -> p h d", d=65)[:, :, 0:64]
                    nc.vector.tensor_copy(dest, pt[:])

        def attention(q_tiles, k_tiles, v_tiles, o_tiles):
            for h in range(H):
                mi, off = h // 2, 64 * (h % 2)
                op = po.tile([65, 512], f32, tag="po")
                for jp in range(4):
                    st = ps.tile([128, 1024], f32, tag="ps")
                    for half in range(2):
                        kj = 2 * jp + half
                        nc.tensor.matmul(
                            st[:, half * 512:(half + 1) * 512],
                            k_tiles[mi][off:off + 64, kj * 128:(kj + 1) * 128],
                            q_tiles[mi][off:off + 64, :],
                            start=True, stop=True)
                    # additive causal band on the diagonal-straddling 64 queries
                    c0 = 64 * (2 * jp)
                    nc.vector.tensor_add(st[:, c0:c0 + 64], st[:, c0:c0 + 64], band_t[:])
                    c1 = 512 + 64 * (2 * jp + 1)
                    nc.vector.tensor_add(st[:, c1:c1 + 64], st[:, c1:c1 + 64], band_t[:])
                    et = epool.tile([128, 1024], bf16, tag="epool")
                    nc.scalar.activation(et[:], st[:], AF.Exp)
                    # zero fully-masked rectangles (queries strictly before key block)
                    if jp > 0:
                        nc.gpsimd.memset(et[:, 0:64 * 2 * jp], 0.0)
                    nc.gpsimd.memset(et[:, 512:512 + 64 * (2 * jp + 1)], 0.0)
                    for half in range(2):
                        kj = 2 * jp + half
                        nc.tensor.matmul(
                            op[:], v_tiles[kj][:, 65 * h:65 * h + 65],
                            et[:, half * 512:(half + 1) * 512],
                            start=(jp == 0 and half == 0), stop=(jp == 3 and half == 1))
                rr = rrows.tile([1, 512], f32, tag="rrows")
                nc.vector.reciprocal(rr[:], op[64:65, :])
                bp = pp.tile([64, 512], f32, tag="pp")
                nc.tensor.matmul(bp[:], ones_r32[0:1, 0:64], rr[:], start=True, stop=True)
                rbc = recb.tile([64, 512], f32, tag="recb")
                nc.vector.tensor_copy(rbc[:], bp[:])
                nc.vector.tensor_mul(o_tiles[mi][off:off + 64, :], op[0:64, :], rbc[:])

        # =============== phase 1: LN1 over full x (2-pass) + own x ===============
        mean_ps = ps.tile([1, T], f32, tag="ps")
        sq_ps = ps.tile([1, T], f32, tag="ps")
        for k in range(NT):
            xt = big32.tile([128, T], f32, tag="big32")
            nc.sync.dma_start(xt[:], xT_d[k * 128:(k + 1) * 128, :])
            sq = sqpool.tile([128, T], bf16, tag="sqf")
            nc.vector.tensor_mul(sq[:], xt[:], xt[:])
            for hh in range(2):
                sl = slice(hh * 512, (hh + 1) * 512)
                nc.tensor.matmul(mean_ps[0:1, sl], ones_c32[:], xt[:, sl],
                                 start=(k == 0), stop=(k == NT - 1))
                nc.tensor.matmul(sq_ps[0:1, sl], ones_c16[:], sq[:, sl],
                                 start=(k == 0), stop=(k == NT - 1))
        mean_row = rows.tile([1, T], f32, tag="rows")
        rstd_row = rows.tile([1, T], f32, tag="rows")
        nc.vector.tensor_scalar_mul(mean_row[:], mean_ps[:], 1.0 / C)
        nc.vector.tensor_mul(rstd_row[:], mean_row[:], mean_row[:])
        nc.vector.scalar_tensor_tensor(rstd_row[:], sq_ps[:], 1.0 / C, rstd_row[:],
                                       op0=AL.mult, op1=AL.subtract)
        nc.scalar.activation(rstd_row[:], rstd_row[:], AF.Sqrt, bias=eps_t[:])
        nc.vector.reciprocal(rstd_row[:], rstd_row[:])
        mb_f = sbig.tile([128, T], f32, tag="sbig")
        rb_f = sbig.tile([128, T], f32, tag="sbig")
        bcast_row(mean_row, mb_f, T)
        bcast_row(rstd_row, rb_f, T)
        lnb = [acts.tile([128, T], bf16, tag="acts", name=f"lnb{i}") for i in range(NT)]
        for k in range(NT):
            xt = big32.tile([128, T], f32, tag="big32")
            nc.sync.dma_start(xt[:], xT_d[k * 128:(k + 1) * 128, :])
            nc.vector.tensor_sub(xt[:], xt[:], mb_f[:])
            nc.vector.tensor_mul(xt[:], xt[:], rb_f[:])
            nc.scalar.activation(lnb[k][:], xt[:], AF.Identity,
                                 bias=bias_t["b1"][:, k:k + 1], scale=bias_t["g1"][:, k:k + 1])

        # own-token x -> residual tiles + LN(own)
        resid = []
        for k in range(NT):
            rt = rpool.tile([128, TQ], f32, tag="rpool")
            nc.sync.dma_start(rt[:], xqT_d[k * 128:(k + 1) * 128, :])
            resid.append(rt)
        lnown = [lnsm.tile([128, TQ], bf16, tag="lnsm", name=f"lnown{i}") for i in range(NT)]
        ln_stats_and_norm(resid, bias_t["g1"], bias_t["b1"], lnown)

        # =============== phase 2: self qkv ===============
        a_sa_t = load_lora_a("a_sa")
        z_sa = compute_z(a_sa_t, lnb, T, "zbig")
        z_own = compute_z(a_sa_t, lnown, TQ, "zsm")

        qT = [qpool.tile([128, TQ], bf16, tag="qpool", name=f"qT{i}") for i in range(NT)]

        def q_cb(mi, pt, h):
            nc.scalar.activation(qT[mi][:], pt[:], AF.Identity,
                                 bias=bias_t["bq"][:, mi:mi + 1])

        projT("wq", lnown, TQ, z_own, "b_saq", q_cb)

        kT = [kpool.tile([128, T], bf16, tag="kpool", name=f"kT{i}") for i in range(NT)]

        def k_cb(mi, pt, h):
            nc.scalar.activation(kT[mi][:, h * 512:(h + 1) * 512], pt[:], AF.Identity,
                                 bias=bias_t["bk"][:, mi:mi + 1])

        projT("wk", lnb, T, z_sa, "b_sak", k_cb)

        vt = [vpool.tile([128, 1040], bf16, tag="vpool", name=f"vt{i}") for i in range(NT)]
        for tt in range(NT):
            nc.gpsimd.memset(vt[tt][:, 64:1040:65], 1.0)
        proj_V("wv", lnb, z_sa, bv_t, "b_sav", vt)

        # =============== phase 3: cross K (PE filler during self-attn) ===============
        fb = [acts.tile([128, T], bf16, tag="acts", name=f"fb{i}") for i in range(NT)]
        for k in range(NT):
            ft = big32.tile([128, T], f32, tag="big32")
            nc.sync.dma_start(ft[:], fT_d[k * 128:(k + 1) * 128, :])
            nc.vector.tensor_copy(fb[k][:], ft[:])
        a_ck_t = load_lora_a("a_ck")
        z_ck = compute_z(a_ck_t, fb, T, "zbig2")
        k2T = [k2pool.tile([128, T], bf16, tag="k2pool", name=f"k2T{i}") for i in range(NT)]

        def k2_cb(mi, pt, h):
            nc.scalar.activation(k2T[mi][:, h * 512:(h + 1) * 512], pt[:], AF.Identity,
                                 bias=bias_t["bck"][:, mi:mi + 1])

        projT("wck", fb, T, z_ck, "b_ckk", k2_cb)

        # =============== phase 4: self attention ===============
        oT = [opool.tile([128, TQ], bf16, tag="opool", name=f"oT{i}") for i in range(NT)]
        attention(qT, kT, vt, oT)

        # =============== phase 5: cross V (reuses V slots) ===============
        v2t = [vpool.tile([128, 1040], bf16, tag="vpool", name=f"v2t{i}") for i in range(NT)]
        for tt in range(NT):
            nc.gpsimd.memset(v2t[tt][:, 64:1040:65], 1.0)
        proj_V("wcv", fb, z_ck, bcv_t, "b_ckv", v2t)

        # =============== phase 6: self proj + residual ===============
        a_sp_t = load_lora_a("a_sp")
        z_sp = compute_z(a_sp_t, oT, TQ, "zsm")

        def sp_cb(mi, pt, h):
            nc.vector.scalar_tensor_tensor(resid[mi][:], pt[:], bias_t["bsp"][:, mi:mi + 1],
                                           resid[mi][:], op0=AL.add, op1=AL.add)

        projT("wsp", oT, TQ, z_sp, "b_sp", sp_cb)

        # =============== phase 7: LN1 on updated own tokens ===============
        ln1b = [lnsm.tile([128, TQ], bf16, tag="lnsm", name=f"ln1b{i}") for i in range(NT)]
        ln_stats_and_norm(resid, bias_t["g1"], bias_t["b1"], ln1b)

        # =============== phase 8: cross q ===============
        a_cq_t = load_lora_a("a_cq")
        z_cq = compute_z(a_cq_t, ln1b, TQ, "zsm")
        q2T = [qpool.tile([128, TQ], bf16, tag="qpool", name=f"q2T{i}") for i in range(NT)]

        def q2_cb(mi, pt, h):
            nc.scalar.activation(q2T[mi][:], pt[:], AF.Identity,
                                 bias=bias_t["bcq"][:, mi:mi + 1])

        projT("wcq", ln1b, TQ, z_cq, "b_cq", q2_cb)

        # =============== phase 9: cross attention ===============
        o2T = [opool.tile([128, TQ], bf16, tag="opool", name=f"o2T{i}") for i in range(NT)]
        attention(q2T, k2T, v2t, o2T)

        # =============== phase 10: cross proj + residual ===============
        a_cp_t = load_lora_a("a_cp")
        z_cp = compute_z(a_cp_t, o2T, TQ, "zsm")

        def cp_cb(mi, pt, h):
            nc.vector.scalar_tensor_tensor(resid[mi][:], pt[:], bias_t["bcp"][:, mi:mi + 1],
                                           resid[mi][:], op0=AL.add, op1=AL.add)

        projT("wcp", o2T, TQ, z_cp, "b_cp", cp_cb)

        # =============== phase 11: LN2 + MLP (per token-half) ===============
        ln2 = [lnsm.tile([128, TQ], bf16, tag="lnsm", name=f"ln2_{i}") for i in range(NT)]
        ln_stats_and_norm(resid, bias_t["g2"], bias_t["b2"], ln2)

        for th in range(2):
            tsl = slice(th * 256, (th + 1) * 256)
            m_sb = [None] * 32
            for grp in range(8):
                wts = []
                for k in range(NT):
                    wt = wpool.tile([128, 512], bf16, tag="wpool")
                    nc.sync.dma_start(wt[:], w_d["wfc"][k * 128:(k + 1) * 128,
                                                        grp * 512:(grp + 1) * 512])
                    wts.append(wt)
                for ml in range(4):
                    mi = grp * 4 + ml
                    pt = pp.tile([128, 256], f32, tag="pp")
                    for k in range(NT):
                        nc.tensor.matmul(pt[:], wts[k][:, ml * 128:(ml + 1) * 128],
                                         ln2[k][:, tsl], start=(k == 0), stop=(k == NT - 1))
                    mt = mpool.tile([128, 256], bf16, tag="mpool")
                    nc.scalar.activation(mt[:], pt[:], AF.Gelu_apprx_tanh,
                                         bias=bias_t["bfc"][:, mi:mi + 1])
                    m_sb[mi] = mt

            for quad in range(2):
                qts = []
                for j in range(4):
                    p_ = ps if j < 2 else po
                    qts.append(p_.tile([128, 256], f32, tag="ps" if j < 2 else "po", name=f"prq{th}_{quad}_{j}"))
                for k in range(32):
                    wt = wpool.tile([128, 512], bf16, tag="wpool")
                    nc.sync.dma_start(wt[:], w_d["wpr"][k * 128:(k + 1) * 128,
                                                        quad * 512:(quad + 1) * 512])
                    for j in range(4):
                        nc.tensor.matmul(qts[j][:], wt[:, j * 128:(j + 1) * 128],
                                         m_sb[k][:], start=(k == 0), stop=(k == 31))
                for j in range(4):
                    mi = quad * 4 + j
                    of = outfp.tile([128, 256], f32, tag="outfp")
                    nc.vector.scalar_tensor_tensor(of[:], qts[j][:],
                                                   bias_t["bpr"][:, mi:mi + 1],
                                                   resid[mi][:, tsl],
                                                   op0=AL.add, op1=AL.add)
                    nc.sync.dma_start(outT_d[mi * 128:(mi + 1) * 128, tsl], of[:])

    nc.compile()
    return nc


def _get_program():
    global _PROG
    if _PROG is None:
        _PROG = _build_program()
    return _PROG


def _prep_shared(inputs):
    g = {}

    def bf(a):
        return np.ascontiguousarray(np.asarray(a, dtype=np.float32)).astype(BF)

    def f(a):
        return np.ascontiguousarray(np.asarray(a, dtype=np.float32))

    qw, kw, vw = (inputs["sa_qkv_w"][i * C:(i + 1) * C] for i in range(3))
    qb, kb, vb = (inputs["sa_qkv_b"][i * C:(i + 1) * C] for i in range(3))
    qlb, klb, vlb = (inputs["sa_qkv_lb"][i * C:(i + 1) * C] for i in range(3))
    inv = 1.0 / np.sqrt(DH)
    g["wq"] = bf(np.asarray(qw).T * inv)
    g["wk"] = bf(np.asarray(kw).T)
    g["wv"] = bf(np.asarray(vw).T)
    g["bq"] = f(np.asarray(qb) * inv)
    g["bk"] = f(kb)
    g["bv_row"] = bf(np.asarray(vb).reshape(1, C))
    g["a_sa"] = bf(np.asarray(inputs["sa_qkv_a"]).T)
    g["b_saq"] = bf(np.asarray(qlb).T * (SCALE * inv))
    g["b_sak"] = bf(np.asarray(klb).T * SCALE)
    g["b_sav"] = bf(np.asarray(vlb).T * SCALE)

    g["wsp"] = bf(np.asarray(inputs["sa_proj_w"]).T)
    g["bsp"] = f(inputs["sa_proj_b"])
    g["a_sp"] = bf(np.asarray(inputs["sa_proj_a"]).T)
    g["b_sp"] = bf(np.asarray(inputs["sa_proj_lb"]).T * SCALE)

    g["wcq"] = bf(np.asarray(inputs["ca_q_w"]).T * inv)
    g["bcq"] = f(np.asarray(inputs["ca_q_b"]) * inv)
    g["a_cq"] = bf(np.asarray(inputs["ca_q_a"]).T)
    g["b_cq"] = bf(np.asarray(inputs["ca_q_lb"]).T * (SCALE * inv))

    ckw, cvw = inputs["ca_kv_w"][0:C], inputs["ca_kv_w"][C:2 * C]
    ckb, cvb = inputs["ca_kv_b"][0:C], inputs["ca_kv_b"][C:2 * C]
    cklb, cvlb = inputs["ca_kv_lb"][0:C], inputs["ca_kv_lb"][C:2 * C]
    g["wck"] = bf(np.asarray(ckw).T)
    g["wcv"] = bf(np.asarray(cvw).T)
    g["bck"] = f(ckb)
    g["bcv_row"] = bf(np.asarray(cvb).reshape(1, C))
    g["a_ck"] = bf(np.asarray(inputs["ca_kv_a"]).T)
    g["b_ckk"] = bf(np.asarray(cklb).T * SCALE)
    g["b_ckv"] = bf(np.asarray(cvlb).T * SCALE)

    g["wcp"] = bf(np.asarray(inputs["ca_proj_w"]).T)
    g["bcp"] = f(inputs["ca_proj_b"])
    g["a_cp"] = bf(np.asarray(inputs["ca_proj_a"]).T)
    g["b_cp"] = bf(np.asarray(inputs["ca_proj_lb"]).T * SCALE)

    g["wfc"] = bf(np.asarray(inputs["fc_w"]).T)
    g["bfc"] = f(inputs["fc_b"])
    g["wpr"] = bf(np.asarray(inputs["pr_w"]).T)
    g["bpr"] = f(inputs["pr_b"])
    g["g1"] = f(inputs["ln1_g"])
    g["b1"] = f(inputs["ln1_b"])
    g["g2"] = f(inputs["ln2_g"])
    g["b2"] = f(inputs["ln2_b"])
    return g


def _make_in_maps(inputs):
    inputs = {k: np.asarray(v) for k, v in inputs.items()}
    x, feat = inputs["x"], inputs["feature"]
    B = x.shape[0]
    shared = _prep_shared(inputs)

    bands = []
    for p in range(2):
        jj = np.arange(128).reshape(128, 1)
        ii = np.arange(64).reshape(1, 64)
        bands.append(np.where(jj <= 2 * ii + p, 0.0, -10000.0).astype(np.float32))

    in_maps = []
    xTs = [np.ascontiguousarray(np.asarray(x[b]).T, dtype=np.float32) for b in range(B)]
    fTs = [np.ascontiguousarray(np.asarray(feat[b]).T, dtype=np.float32) for b in range(B)]
    for core in range(NCORES):
        b, p = core // 2, core % 2
        m = dict(shared)
        m["xT"] = xTs[b]
        m["xqT"] = np.ascontiguousarray(xTs[b][:, p::2])
        m["fT"] = fTs[b]
        m["band"] = bands[p]
        in_maps.append(m)
    return in_maps, B


def kernel(**inputs):
    from concourse.bass_utils import run_bass_kernel_spmd

    nc = _get_program()
    in_maps, B = _make_in_maps(inputs)
    res = run_bass_kernel_spmd(nc, in_maps, core_ids=list(range(NCORES)))
    out = np.zeros((B, T, C), np.float32)
    for core in range(NCORES):
        b, p = core // 2, core % 2
        out[b, p::2, :] = np.asarray(res.results[core]["outT"], dtype=np.float32).T
    return out


# revision 12
# speedup vs baseline: 1.2085x; 1.2085x over previous
"""Trainium2 Bass kernel for nn_Block_with_lora (dense transformer block).

Sharding: 8 cores = 4 batches x 2 token-parity shards (stride-2 over T).
Each core computes its 512 query tokens end-to-end (no collectives);
K/V projections over all 1024 tokens are computed per-core (uniform SPMD
program; all batch/parity dependence lives in the per-core input data).

Layout: all activations transposed [C, T] (host transposes I/O), so every
projection is a natural PE matmul. Attention uses S^T = K^T.T @ Q^T tiles
[tk, tq]; softmax denominator rides the AV matmul as an extra ones-column
of V; masking = additive diagonal band (DVE) + rectangle memsets (GPSIMD).
"""

import sys

sys.path.insert(0, "/opt/trn_rl_repo")

import numpy as np
import ml_dtypes
from contextlib import ExitStack

BF = ml_dtypes.bfloat16

C = 1024
H = 16
DH = 64
R = 16
SCALE = 1.0 / R
T = 1024
TQ = 512
NT = 8  # C / 128
EPS = 1e-5
NCORES = 8

_PROG = None


def _build_program():
    import concourse.bass as bass
    import concourse.tile as tile
    from concourse import mybir, bacc

    f32 = mybir.dt.float32
    bf16 = mybir.dt.bfloat16
    AF = mybir.ActivationFunctionType
    AL = mybir.AluOpType

    nc = bacc.Bacc("TRN2", target_bir_lowering=False, debug=False)

    def din(name, shape, dt=f32):
        return nc.dram_tensor(name, shape, dt, kind="ExternalInput").ap()

    xT_d = din("xT", [C, T])
    xqT_d = din("xqT", [C, TQ])
    fT_d = din("fT", [C, T])
    band_d = din("band", [128, 64])

    w_d = {}
    for n in ["wq", "wk", "wv", "wsp", "wcq", "wck", "wcv", "wcp"]:
        w_d[n] = din(n, [C, C], bf16)
    w_d["wfc"] = din("wfc", [C, 4 * C], bf16)
    w_d["wpr"] = din("wpr", [4 * C, C], bf16)
    a_d = {n: din(n, [C, R], bf16) for n in ["a_sa", "a_sp", "a_cq", "a_ck", "a_cp"]}
    b_d = {
        n: din(n, [R, C], bf16)
        for n in ["b_saq", "b_sak", "b_sav", "b_sp", "b_cq", "b_ckk", "b_ckv", "b_cp"]
    }
    bias_d = {
        n: din(n, [C], f32)
        for n in ["bq", "bk", "bsp", "bcq", "bck", "bcp", "bpr", "g1", "b1", "g2", "b2"]
    }
    bias_d["bfc"] = din("bfc", [4 * C], f32)
    bvrow_d = din("bv_row", [1, C], bf16)
    sel_d = din("sel", [NT, R, 128], f32)
    bcvrow_d = din("bcv_row", [1, C], bf16)

    outT_d = nc.dram_tensor("outT", [C, TQ], f32, kind="ExternalOutput").ap()

    with tile.TileContext(nc) as tc, ExitStack() as ctx:

        def pool(name, bufs, space=None):
            kw = dict(name=name, bufs=bufs)
            if space:
                kw["space"] = space
            return ctx.enter_context(tc.tile_pool(**kw))

        # SBUF pools (budget ~181KB/partition of 192)
        big32 = pool("big32", 3)        # [128,1024] f32: x/f stream + LN temps
        acts = pool("acts", 8)          # [128,1024] bf16: lnb then fb
        lnsm = pool("lnsm", 8)          # [128,512] bf16: lnown -> ln1b -> ln2
        qpool = pool("qpool", 8)        # [128,512] bf16: qT -> q2T
        kpool = pool("kpool", 8)        # [128,1024] bf16: kT
        k2pool = pool("k2pool", 8)      # [128,1024] bf16: k2T (separate: overlaps attn)
        vpool = pool("vpool", 8)        # [128,1040] bf16: V -> V2
        opool = pool("opool", 8)        # [128,512] bf16: oT -> o2T
        rpool = pool("rpool", 8)        # [128,512] f32: residual (persist)
        mpool = pool("mpool", 32)       # [128,256] bf16: MLP hidden (per t-half)
        wpool = pool("wpool", 10)        # [128,512] bf16: weight chunks
        epool = pool("epool", 3)        # [128,1024] bf16: exp(S)
        sqpool = pool("sqpool", 2)      # squares for LN var
        sbig = pool("sbig", 2)          # [128,1024] f32: LN mean/rstd bcast
        rows = pool("rows", 2)          # [1,1024] f32: LN stat rows
        rrows = pool("rrows", 2)        # [1,512] f32: softmax recip rows
        recb = pool("recb", 2)          # [64,512] f32: recip bcast
        dallp = pool("dallp", 2)        # [16,512] f32: batched softmax denoms
        outfp = pool("outfp", 2)        # [128,256] f32: final out staging
        zpool = pool("zpool", 1)        # [16,*] bf16: lora z (1 slot per tag)
        lorab = pool("lorab", 1)        # [16,1024] bf16: lora B rows
        loraa = pool("loraa", 10)       # [128,16] bf16: lora A chunks
        smalls = pool("smalls", 1)      # [128,<=32] bias/g/b columns (per tag)
        onesp = pool("onesp", 1)
        bandp = pool("bandp", 1)
        bvp = pool("bvp", 1)            # [1,1024] bf16 v-bias rows

        # PSUM pools: 4 + 2 + 2 = 8 banks
        ps = pool("ps", 2, space="PSUM")   # [128,1024] f32: S tiles, LN stats, pr acc
        po = pool("po", 2, space="PSUM")   # [65..128,512] f32: attn out acc, pr acc
        pp = pool("pp", 2, space="PSUM")   # [128,512] f32: projections, z

        # ---- constants ----
        ones_c32 = onesp.tile([128, 1], f32, tag="oc32")
        nc.gpsimd.memset(ones_c32[:], 1.0)
        ones_c16 = onesp.tile([128, 1], bf16, tag="oc16")
        nc.gpsimd.memset(ones_c16[:], 1.0)
        ones_r16 = onesp.tile([1, 128], bf16, tag="or16")
        nc.gpsimd.memset(ones_r16[:], 1.0)
        ones_r32 = onesp.tile([1, 128], f32, tag="or32")
        nc.gpsimd.memset(ones_r32[:], 1.0)

        band_t = bandp.tile([128, 64], f32, tag="band")
        nc.sync.dma_start(band_t[:], band_d[:, :])
        # selector matrices: sel[mi] @ dall broadcasts head 2mi to rows 0:64
        # and head 2mi+1 to rows 64:128 (softmax denominator rescale)
        sel_t = []
        for mi in range(NT):
            st_ = smalls.tile([R, 128], f32, tag=f"sel{mi}", name=f"sel{mi}")
            nc.sync.dma_start(st_[:], sel_d[mi])
            sel_t.append(st_)
        eps_t = onesp.tile([1, 1], f32, tag="eps")
        nc.gpsimd.memset(eps_t[:], EPS)

        def load_percol(name, n=NT):
            t = smalls.tile([128, n], f32, tag=name)
            nc.sync.dma_start(t[:], bias_d[name].rearrange("(m p) -> p m", p=128))
            return t

        bias_t = {
            n: load_percol(n)
            for n in ["bq", "bk", "bsp", "bcq", "bcp", "bpr", "g1", "b1", "g2", "b2", "bck"]
        }
        bias_t["bfc"] = load_percol("bfc", 32)
        bv_t = bvp.tile([1, C], bf16, tag="bv")
        nc.sync.dma_start(bv_t[:], bvrow_d[:, :])
        bcv_t = bvp.tile([1, C], bf16, tag="bcv")
        nc.sync.dma_start(bcv_t[:], bcvrow_d[:, :])

        def load_lora_a(name):
            ts = []
            for k in range(NT):
                t = loraa.tile([128, R], bf16, tag="loraa")
                nc.sync.dma_start(t[:], a_d[name][k * 128:(k + 1) * 128, :])
                ts.append(t)
            return ts

        def load_lora_b(name):
            t = lorab.tile([R, C], bf16, tag="lorab")
            nc.sync.dma_start(t[:], b_d[name][:, :])
            return t

        # =============== helpers ===============
        def bcast_row(row, out_sb, Tn):
            # broadcast [1, Tn] f32 row to [128, Tn] SBUF via K=1 PE matmul
            for h in range(Tn // 512):
                sl = slice(h * 512, (h + 1) * 512)
                bp = pp.tile([128, 512], f32, tag="pp")
                nc.tensor.matmul(bp[:], ones_r32[:], row[0:1, sl], start=True, stop=True)
                nc.vector.tensor_copy(out_sb[:, sl], bp[:])

        def ln_stats_and_norm(src_tiles, g_col, b_col, out_tiles):
            """LayerNorm over channel (partition) dim; src 8x[128,512] f32 persistent."""
            mean_ps = ps.tile([1, TQ], f32, tag="ps")
            sq_ps = ps.tile([1, TQ], f32, tag="ps")
            for k in range(NT):
                sq = sqpool.tile([128, TQ], bf16, tag="sqo")
                nc.vector.tensor_mul(sq[:], src_tiles[k][:], src_tiles[k][:])
                nc.tensor.matmul(mean_ps[:], ones_c32[:], src_tiles[k][:],
                                 start=(k == 0), stop=(k == NT - 1))
                nc.tensor.matmul(sq_ps[:], ones_c16[:], sq[:],
                                 start=(k == 0), stop=(k == NT - 1))
            mean_row = rows.tile([1, TQ], f32, tag="rows")
            rstd_row = rows.tile([1, TQ], f32, tag="rows")
            nc.vector.tensor_scalar_mul(mean_row[:], mean_ps[:], 1.0 / C)
            nc.vector.tensor_mul(rstd_row[:], mean_row[:], mean_row[:])
            nc.vector.scalar_tensor_tensor(rstd_row[:], sq_ps[:], 1.0 / C, rstd_row[:],
                                           op0=AL.mult, op1=AL.subtract)
            nc.scalar.activation(rstd_row[:], rstd_row[:], AF.Sqrt, bias=eps_t[:])
            nc.vector.reciprocal(rstd_row[:], rstd_row[:])
            mb = sbig.tile([128, TQ], f32, tag="sbig")
            rb = sbig.tile([128, TQ], f32, tag="sbig")
            bcast_row(mean_row, mb, TQ)
            bcast_row(rstd_row, rb, TQ)
            for k in range(NT):
                t1 = big32.tile([128, TQ], f32, tag="big32")
                nc.vector.tensor_sub(t1[:], src_tiles[k][:], mb[:])
                nc.vector.tensor_mul(t1[:], t1[:], rb[:])
                nc.scalar.activation(out_tiles[k][:], t1[:], AF.Identity,
                                     bias=b_col[:, k:k + 1], scale=g_col[:, k:k + 1])

        def compute_z(a_tiles, rhs_tiles, Tn, tag):
            """z^T = A-proj of activations: [16, Tn] bf16."""
            z_sb = zpool.tile([R, Tn], bf16, tag=tag)
            for h in range(Tn // 512):
                sl = slice(h * 512, (h + 1) * 512)
                zp = pp.tile([R, 512], f32, tag="pp")
                for k in range(NT):
                    nc.tensor.matmul(zp[:], a_tiles[k][:], rhs_tiles[k][:, sl],
                                     start=(k == 0), stop=(k == NT - 1))
                nc.vector.tensor_copy(z_sb[:, sl], zp[:])
            return z_sb

        def projT(wname, rhs_tiles, Tn, z_sb, bname, out_cb, pools=None):
            """out^T tiles via PE; lora + callback per (M-tile, t-half) psum."""
            if pools is None:
                pools = ((pp, "pp"),)
            b_t = load_lora_b(bname)
            pcnt = 0
            for mh in range(2):  # c_out halves of 512
                wts = []
                for k in range(NT):
                    wt = wpool.tile([128, 512], bf16, tag="wpool")
                    nc.sync.dma_start(wt[:], w_d[wname][k * 128:(k + 1) * 128,
                                                        mh * 512:(mh + 1) * 512])
                    wts.append(wt)
                for ml in range(4):
                    mi = mh * 4 + ml
                    for h in range(Tn // 512):
                        sl = slice(h * 512, (h + 1) * 512)
                        pl, ptag = pools[pcnt % len(pools)]
                        pcnt += 1
                        pt = pl.tile([128, 512], f32, tag=ptag)
                        for k in range(NT):
                            nc.tensor.matmul(pt[:], wts[k][:, ml * 128:(ml + 1) * 128],
                                             rhs_tiles[k][:, sl], start=(k == 0), stop=False)
                        nc.tensor.matmul(pt[:], b_t[:, mi * 128:(mi + 1) * 128],
                                         z_sb[:, sl], start=False, stop=True)
                        out_cb(mi, pt, h)

        def proj_V(wname, lhs_tiles, z_sb, bv_row_t, bname, v_tiles, pools=None):
            """V natural [t, d] with activations stationary; +lora +bias(ones-MM)."""
            if pools is None:
                pools = ((pp, "pp"),)
            b_t = load_lora_b(bname)
            pcnt = 0
            for dh in range(2):
                sl = slice(dh * 512, (dh + 1) * 512)
                wts = []
                for k in range(NT):
                    wt = wpool.tile([128, 512], bf16, tag="wpool")
                    nc.sync.dma_start(wt[:], w_d[wname][k * 128:(k + 1) * 128, sl])
                    wts.append(wt)
                for tt in range(NT):
                    pl, ptag = pools[pcnt % len(pools)]
                    pcnt += 1
                    pt = pl.tile([128, 512], f32, tag=ptag)
                    for k in range(NT):
                        nc.tensor.matmul(pt[:], lhs_tiles[k][:, tt * 128:(tt + 1) * 128],
                                         wts[k][:], start=(k == 0), stop=False)
                    nc.tensor.matmul(pt[:], z_sb[:, tt * 128:(tt + 1) * 128],
                                     b_t[:, sl], start=False, stop=False)
                    nc.tensor.matmul(pt[:], ones_r16[:], bv_row_t[:, sl],
                                     start=False, stop=True)
                    dest = v_tiles[tt][:, dh * 520:(dh + 1) * 520]
                    dest = dest.rearrange("p (h d) -> p h d", d=65)[:, :, 0:64]
                    nc.vector.tensor_copy(dest, pt[:])

        def attention(q_tiles, k_tiles, v_tiles, o_tiles):
            # Block kj only matters for queries i >= 64*kj (strided parity
            # layout), so every matmul/exp runs on the live tq-subrange --
            # no fully-masked rectangles to zero.
            dall = dallp.tile([R, 512], f32, tag="dallp")
            for h in range(H):
                mi, off = h // 2, 64 * (h % 2)
                op = po.tile([65, 512], f32, tag="po")
                for jp in range(4):
                    st = ps.tile([128, 1024], f32, tag="ps")
                    for half in range(2):
                        kj = 2 * jp + half
                        q0 = 64 * kj
                        base = half * 512
                        nc.tensor.matmul(
                            st[:, base + q0:base + 512],
                            k_tiles[mi][off:off + 64, kj * 128:(kj + 1) * 128],
                            q_tiles[mi][off:off + 64, q0:512],
                            start=True, stop=True)
                        # additive causal band on the diagonal-straddling queries
                        nc.vector.tensor_add(st[:, base + q0:base + q0 + 64],
                                             st[:, base + q0:base + q0 + 64], band_t[:])
                    et = epool.tile([128, 1024], bf16, tag="epool")
                    for half in range(2):
                        kj = 2 * jp + half
                        q0 = 64 * kj
                        base = half * 512
                        nc.scalar.activation(et[:, base + q0:base + 512],
                                             st[:, base + q0:base + 512], AF.Exp)
                        nc.tensor.matmul(
                            op[:, q0:512] if kj > 0 else op[:],
                            v_tiles[kj][:, 65 * h:65 * h + 65],
                            et[:, base + q0:base + 512],
                            start=(kj == 0), stop=(kj == 7))
                # stash raw (unnormalized) head output + its denominator row
                nc.vector.tensor_copy(o_tiles[mi][off:off + 64, :], op[0:64, :])
                rr = rrows.tile([1, 512], f32, tag="rrows")
                nc.vector.tensor_copy(rr[:], op[64:65, :])
                nc.sync.dma_start(dall[h:h + 1, :], rr[:])
            # one batched reciprocal for all 16 heads, then per-tile rescale
            nc.vector.reciprocal(dall[:], dall[:])
            for mi2 in range(NT):
                bp = pp.tile([128, 512], f32, tag="pp")
                nc.tensor.matmul(bp[:], sel_t[mi2][:], dall[:], start=True, stop=True)
                rbc = recb.tile([128, 512], f32, tag="recb")
                nc.vector.tensor_copy(rbc[:], bp[:])
                nc.vector.tensor_mul(o_tiles[mi2][:], o_tiles[mi2][:], rbc[:])

        # =============== phase 1: LN1 over full x (2-pass) + own x ===============
        mean_ps = ps.tile([1, T], f32, tag="ps")
        sq_ps = ps.tile([1, T], f32, tag="ps")
        for k in range(NT):
            xt = big32.tile([128, T], f32, tag="big32")
            nc.sync.dma_start(xt[:], xT_d[k * 128:(k + 1) * 128, :])
            sq = sqpool.tile([128, T], bf16, tag="sqf")
            nc.vector.tensor_mul(sq[:], xt[:], xt[:])
            for hh in range(2):
                sl = slice(hh * 512, (hh + 1) * 512)
                nc.tensor.matmul(mean_ps[0:1, sl], ones_c32[:], xt[:, sl],
                                 start=(k == 0), stop=(k == NT - 1))
                nc.tensor.matmul(sq_ps[0:1, sl], ones_c16[:], sq[:, sl],
                                 start=(k == 0), stop=(k == NT - 1))
        mean_row = rows.tile([1, T], f32, tag="rows")
        rstd_row = rows.tile([1, T], f32, tag="rows")
        nc.vector.tensor_scalar_mul(mean_row[:], mean_ps[:], 1.0 / C)
        nc.vector.tensor_mul(rstd_row[:], mean_row[:], mean_row[:])
        nc.vector.scalar_tensor_tensor(rstd_row[:], sq_ps[:], 1.0 / C, rstd_row[:],
                                       op0=AL.mult, op1=AL.subtract)
        nc.scalar.activation(rstd_row[:], rstd_row[:], AF.Sqrt, bias=eps_t[:])
        nc.vector.reciprocal(rstd_row[:], rstd_row[:])
        mb_f = sbig.tile([128, T], f32, tag="sbig")
        rb_f = sbig.tile([128, T], f32, tag="sbig")
        bcast_row(mean_row, mb_f, T)
        bcast_row(rstd_row, rb_f, T)
        lnb = [acts.tile([128, T], bf16, tag="acts", name=f"lnb{i}") for i in range(NT)]
        for k in range(NT):
            xt = big32.tile([128, T], f32, tag="big32")
            nc.sync.dma_start(xt[:], xT_d[k * 128:(k + 1) * 128, :])
            nc.vector.tensor_sub(xt[:], xt[:], mb_f[:])
            nc.vector.tensor_mul(xt[:], xt[:], rb_f[:])
            nc.scalar.activation(lnb[k][:], xt[:], AF.Identity,
                                 bias=bias_t["b1"][:, k:k + 1], scale=bias_t["g1"][:, k:k + 1])

        # own-token x -> residual tiles + LN(own)
        resid = []
        for k in range(NT):
            rt = rpool.tile([128, TQ], f32, tag="rpool")
            nc.sync.dma_start(rt[:], xqT_d[k * 128:(k + 1) * 128, :])
            resid.append(rt)
        lnown = [lnsm.tile([128, TQ], bf16, tag="lnsm", name=f"lnown{i}") for i in range(NT)]
        ln_stats_and_norm(resid, bias_t["g1"], bias_t["b1"], lnown)

        # =============== phase 2: self qkv ===============
        a_sa_t = load_lora_a("a_sa")
        z_sa = compute_z(a_sa_t, lnb, T, "zbig")
        z_own = compute_z(a_sa_t, lnown, TQ, "zsm")

        qT = [qpool.tile([128, TQ], bf16, tag="qpool", name=f"qT{i}") for i in range(NT)]

        def q_cb(mi, pt, h):
            nc.scalar.activation(qT[mi][:], pt[:], AF.Identity,
                                 bias=bias_t["bq"][:, mi:mi + 1])

        projT("wq", lnown, TQ, z_own, "b_saq", q_cb, pools=((pp, "pp"), (po, "po")))

        kT = [kpool.tile([128, T], bf16, tag="kpool", name=f"kT{i}") for i in range(NT)]

        def k_cb(mi, pt, h):
            nc.scalar.activation(kT[mi][:, h * 512:(h + 1) * 512], pt[:], AF.Identity,
                                 bias=bias_t["bk"][:, mi:mi + 1])

        projT("wk", lnb, T, z_sa, "b_sak", k_cb, pools=((pp, "pp"), (po, "po")))

        vt = [vpool.tile([128, 1040], bf16, tag="vpool", name=f"vt{i}") for i in range(NT)]
        for tt in range(NT):
            nc.gpsimd.memset(vt[tt][:, 64:1040:65], 1.0)
        proj_V("wv", lnb, z_sa, bv_t, "b_sav", vt, pools=((pp, "pp"), (po, "po")))

        # =============== phase 3: cross K (PE filler during self-attn) ===============
        fb = [acts.tile([128, T], bf16, tag="acts", name=f"fb{i}") for i in range(NT)]
        for k in range(NT):
            ft = big32.tile([128, T], f32, tag="big32")
            nc.sync.dma_start(ft[:], fT_d[k * 128:(k + 1) * 128, :])
            nc.vector.tensor_copy(fb[k][:], ft[:])
        a_ck_t = load_lora_a("a_ck")
        z_ck = compute_z(a_ck_t, fb, T, "zbig2")
        k2T = [k2pool.tile([128, T], bf16, tag="k2pool", name=f"k2T{i}") for i in range(NT)]

        def k2_cb(mi, pt, h):
            nc.scalar.activation(k2T[mi][:, h * 512:(h + 1) * 512], pt[:], AF.Identity,
                                 bias=bias_t["bck"][:, mi:mi + 1])

        projT("wck", fb, T, z_ck, "b_ckk", k2_cb)

        # =============== phase 4: self attention ===============
        oT = [opool.tile([128, TQ], bf16, tag="opool", name=f"oT{i}") for i in range(NT)]
        attention(qT, kT, vt, oT)

        # =============== phase 5: cross V (reuses V slots) ===============
        v2t = [vpool.tile([128, 1040], bf16, tag="vpool", name=f"v2t{i}") for i in range(NT)]
        for tt in range(NT):
            nc.gpsimd.memset(v2t[tt][:, 64:1040:65], 1.0)
        proj_V("wcv", fb, z_ck, bcv_t, "b_ckv", v2t, pools=((pp, "pp"), (po, "po")))

        # =============== phase 6: self proj + residual ===============
        a_sp_t = load_lora_a("a_sp")
        z_sp = compute_z(a_sp_t, oT, TQ, "zsm")

        def sp_cb(mi, pt, h):
            nc.vector.scalar_tensor_tensor(resid[mi][:], pt[:], bias_t["bsp"][:, mi:mi + 1],
                                           resid[mi][:], op0=AL.add, op1=AL.add)

        projT("wsp", oT, TQ, z_sp, "b_sp", sp_cb, pools=((pp, "pp"), (po, "po")))

        # =============== phase 7: LN1 on updated own tokens ===============
        ln1b = [lnsm.tile([128, TQ], bf16, tag="lnsm", name=f"ln1b{i}") for i in range(NT)]
        ln_stats_and_norm(resid, bias_t["g1"], bias_t["b1"], ln1b)

        # =============== phase 8: cross q ===============
        a_cq_t = load_lora_a("a_cq")
        z_cq = compute_z(a_cq_t, ln1b, TQ, "zsm")
        q2T = [qpool.tile([128, TQ], bf16, tag="qpool", name=f"q2T{i}") for i in range(NT)]

        def q2_cb(mi, pt, h):
            nc.scalar.activation(q2T[mi][:], pt[:], AF.Identity,
                                 bias=bias_t["bcq"][:, mi:mi + 1])

        projT("wcq", ln1b, TQ, z_cq, "b_cq", q2_cb, pools=((pp, "pp"), (po, "po")))

        # =============== phase 9: cross attention ===============
        o2T = [opool.tile([128, TQ], bf16, tag="opool", name=f"o2T{i}") for i in range(NT)]
        attention(q2T, k2T, v2t, o2T)

        # =============== phase 10: cross proj + residual ===============
        a_cp_t = load_lora_a("a_cp")
        z_cp = compute_z(a_cp_t, o2T, TQ, "zsm")

        def cp_cb(mi, pt, h):
            nc.vector.scalar_tensor_tensor(resid[mi][:], pt[:], bias_t["bcp"][:, mi:mi + 1],
                                           resid[mi][:], op0=AL.add, op1=AL.add)

        projT("wcp", o2T, TQ, z_cp, "b_cp", cp_cb, pools=((pp, "pp"), (po, "po")))

        # =============== phase 11: LN2 + MLP (per token-half) ===============
        ln2 = [lnsm.tile([128, TQ], bf16, tag="lnsm", name=f"ln2_{i}") for i in range(NT)]
        ln_stats_and_norm(resid, bias_t["g2"], bias_t["b2"], ln2)

        for th in range(2):
            tsl = slice(th * 256, (th + 1) * 256)
            m_sb = [None] * 32
            for grp in range(8):
                wts = []
                for k in range(NT):
                    wt = wpool.tile([128, 512], bf16, tag="wpool")
                    nc.sync.dma_start(wt[:], w_d["wfc"][k * 128:(k + 1) * 128,
                                                        grp * 512:(grp + 1) * 512])
                    wts.append(wt)
                for ml in range(4):
                    mi = grp * 4 + ml
                    pl, ptag = ((pp, "pp"), (ps, "ps"))[ml % 2]
                    pt = pl.tile([128, 256], f32, tag=ptag)
                    for k in range(NT):
                        nc.tensor.matmul(pt[:], wts[k][:, ml * 128:(ml + 1) * 128],
                                         ln2[k][:, tsl], start=(k == 0), stop=(k == NT - 1))
                    mt = mpool.tile([128, 256], bf16, tag="mpool")
                    nc.scalar.activation(mt[:], pt[:], AF.Gelu_apprx_tanh,
                                         bias=bias_t["bfc"][:, mi:mi + 1])
                    m_sb[mi] = mt

            for quad in range(2):
                qts = []
                for j in range(4):
                    p_ = ps if j < 2 else po
                    qts.append(p_.tile([128, 256], f32, tag="ps" if j < 2 else "po", name=f"prq{th}_{quad}_{j}"))
                for k in range(32):
                    wt = wpool.tile([128, 512], bf16, tag="wpool")
                    nc.sync.dma_start(wt[:], w_d["wpr"][k * 128:(k + 1) * 128,
                                                        quad * 512:(quad + 1) * 512])
                    for j in range(4):
                        nc.tensor.matmul(qts[j][:], wt[:, j * 128:(j + 1) * 128],
                                         m_sb[k][:], start=(k == 0), stop=(k == 31))
                for j in range(4):
                    mi = quad * 4 + j
                    of = outfp.tile([128, 256], f32, tag="outfp")
                    nc.vector.scalar_tensor_tensor(of[:], qts[j][:],
                                                   bias_t["bpr"][:, mi:mi + 1],
                                                   resid[mi][:, tsl],
                                                   op0=AL.add, op1=AL.add)
                    nc.sync.dma_start(outT_d[mi * 128:(mi + 1) * 128, tsl], of[:])

    nc.compile()
    return nc


def _get_program():
    global _PROG
    if _PROG is None:
        _PROG = _build_program()
    return _PROG


def _prep_shared(inputs):
    g = {}

    def bf(a):
        return np.ascontiguousarray(np.asarray(a, dtype=np.float32)).astype(BF)

    def f(a):
        return np.ascontiguousarray(np.asarray(a, dtype=np.float32))

    qw, kw, vw = (inputs["sa_qkv_w"][i * C:(i + 1) * C] for i in range(3))
    qb, kb, vb = (inputs["sa_qkv_b"][i * C:(i + 1) * C] for i in range(3))
    qlb, klb, vlb = (inputs["sa_qkv_lb"][i * C:(i + 1) * C] for i in range(3))
    inv = 1.0 / np.sqrt(DH)
    g["wq"] = bf(np.asarray(qw).T * inv)
    g["wk"] = bf(np.asarray(kw).T)
    g["wv"] = bf(np.asarray(vw).T)
    g["bq"] = f(np.asarray(qb) * inv)
    g["bk"] = f(kb)
    g["bv_row"] = bf(np.asarray(vb).reshape(1, C))
    g["a_sa"] = bf(np.asarray(inputs["sa_qkv_a"]).T)
    g["b_saq"] = bf(np.asarray(qlb).T * (SCALE * inv))
    g["b_sak"] = bf(np.asarray(klb).T * SCALE)
    g["b_sav"] = bf(np.asarray(vlb).T * SCALE)

    g["wsp"] = bf(np.asarray(inputs["sa_proj_w"]).T)
    g["bsp"] = f(inputs["sa_proj_b"])
    g["a_sp"] = bf(np.asarray(inputs["sa_proj_a"]).T)
    g["b_sp"] = bf(np.asarray(inputs["sa_proj_lb"]).T * SCALE)

    g["wcq"] = bf(np.asarray(inputs["ca_q_w"]).T * inv)
    g["bcq"] = f(np.asarray(inputs["ca_q_b"]) * inv)
    g["a_cq"] = bf(np.asarray(inputs["ca_q_a"]).T)
    g["b_cq"] = bf(np.asarray(inputs["ca_q_lb"]).T * (SCALE * inv))

    ckw, cvw = inputs["ca_kv_w"][0:C], inputs["ca_kv_w"][C:2 * C]
    ckb, cvb = inputs["ca_kv_b"][0:C], inputs["ca_kv_b"][C:2 * C]
    cklb, cvlb = inputs["ca_kv_lb"][0:C], inputs["ca_kv_lb"][C:2 * C]
    g["wck"] = bf(np.asarray(ckw).T)
    g["wcv"] = bf(np.asarray(cvw).T)
    g["bck"] = f(ckb)
    g["bcv_row"] = bf(np.asarray(cvb).reshape(1, C))
    g["a_ck"] = bf(np.asarray(inputs["ca_kv_a"]).T)
    g["b_ckk"] = bf(np.asarray(cklb).T * SCALE)
    g["b_ckv"] = bf(np.asarray(cvlb).T * SCALE)

    g["wcp"] = bf(np.asarray(inputs["ca_proj_w"]).T)
    g["bcp"] = f(inputs["ca_proj_b"])
    g["a_cp"] = bf(np.asarray(inputs["ca_proj_a"]).T)
    g["b_cp"] = bf(np.asarray(inputs["ca_proj_lb"]).T * SCALE)

    g["wfc"] = bf(np.asarray(inputs["fc_w"]).T)
    g["bfc"] = f(inputs["fc_b"])
    g["wpr"] = bf(np.asarray(inputs["pr_w"]).T)
    g["bpr"] = f(inputs["pr_b"])
    g["g1"] = f(inputs["ln1_g"])
    g["b1"] = f(inputs["ln1_b"])
    g["g2"] = f(inputs["ln2_g"])
    g["b2"] = f(inputs["ln2_b"])
    return g


def _make_in_maps(inputs):
    inputs = {k: np.asarray(v) for k, v in inputs.items()}
    x, feat = inputs["x"], inputs["feature"]
    B = x.shape[0]
    shared = _prep_shared(inputs)

    bands = []
    for p in range(2):
        jj = np.arange(128).reshape(128, 1)
        ii = np.arange(64).reshape(1, 64)
        bands.append(np.where(jj <= 2 * ii + p, 0.0, -10000.0).astype(np.float32))

    sel = np.zeros((NT, R, 128), np.float32)
    for mi in range(NT):
        sel[mi, 2 * mi, 0:64] = 1.0
        sel[mi, 2 * mi + 1, 64:128] = 1.0
    shared["sel"] = sel

    in_maps = []
    xTs = [np.ascontiguousarray(np.asarray(x[b]).T, dtype=np.float32) for b in range(B)]
    fTs = [np.ascontiguousarray(np.asarray(feat[b]).T, dtype=np.float32) for b in range(B)]
    for core in range(NCORES):
        b, p = core // 2, core % 2
        m = dict(shared)
        m["xT"] = xTs[b]
        m["xqT"] = np.ascontiguousarray(xTs[b][:, p::2])
        m["fT"] = fTs[b]
        m["band"] = bands[p]
        in_maps.append(m)
    return in_maps, B


def kernel(**inputs):
    from concourse.bass_utils import run_bass_kernel_spmd

    nc = _get_program()
    in_maps, B = _make_in_maps(inputs)
    res = run_bass_kernel_spmd(nc, in_maps, core_ids=list(range(NCORES)))
    out = np.zeros((B, T, C), np.float32)
    for core in range(NCORES):
        b, p = core // 2, core % 2
        out[b, p::2, :] = np.asarray(res.results[core]["outT"], dtype=np.float32).T
    return out
